# revision 5
# baseline (speedup 1.0000x reference)
"""Trainium2 Bass kernel for nn_Aligner segment_reduce.

Computation: out = (segment_sum(embed_weight[flat_idx]) / lens) @ proj_w + proj_b
Shapes: flat_idx [65536], seg [65536] (sorted), lens [2048],
        embed_weight [50000, 3584], proj_w [3584, 128], proj_b [128].

Strategy (8 NeuronCores, tensor-parallel on d_model):
- Each core owns a 448-wide d_model slice of the embedding table and the
  matching 448 rows of proj_w.
- Vocab is split at 32768 so row indices fit in int16 for the custom
  GPSIMD dma_gather instruction (two gather passes: lo / hi table).
- Tokens are stably sorted by segment id; each gather call fetches 2048
  rows (16 slots x 128 partitions).  A 128-token "column" lands one token
  per partition.
- Per column, an indicator matrix E[tok, seg_in_window] is built on-device
  (is_equal against an iota row matrix) and E.T @ G accumulates segment
  sums for a 128-segment window directly in PSUM.
- Per 128-segment tile: scale by 1/lens (per-partition), transpose via
  matmul with identity, then row-parallel GEMM with the proj_w slice.
- Host sums the 8 partial [2048, 128] outputs and adds proj_b.
"""

import sys

sys.path.insert(0, "/opt/trn_rl_repo")

import numpy as np

T = 65536
B = 2048
V = 50000
D = 3584
DE = 128
NCORES = 8
DSH = D // NCORES          # 448 d_model columns per core
SPLIT = 32768              # vocab split so indices fit int16
VLO, VHI = SPLIT, V - SPLIT
P = 128
CALL_COLS = 16             # 16 columns (2048 indices) per dma_gather call
NSEG_TILES = B // P        # 16
PAD_SEG = -1000            # sentinel seg value for padded token slots

LAST_RESULTS = None        # BassKernelResults of the most recent run


def _ensure_axon_ntff_hook():
    """bass_utils imports antenv.axon_hooks when trace=True under axon;
    some images lack that module.  Provide it, wired to the libaxon ctypes
    NTFF profiler when available (else the hook stays None and bass_utils
    skips tracing gracefully)."""
    try:
        from antenv import axon_hooks  # noqa: F401
        return
    except ImportError:
        pass
    import types

    try:
        import antenv
    except ImportError:
        return
    mod = types.ModuleType("antenv.axon_hooks")
    _hook = [None]
    mod.set_axon_ntff_profile_hook = lambda h: _hook.__setitem__(0, h)
    mod.get_axon_ntff_profile_hook = lambda: _hook[0]
    sys.modules["antenv.axon_hooks"] = mod
    antenv.axon_hooks = mod
    try:
        if "/root/.axon_site" not in sys.path:
            sys.path.insert(0, "/root/.axon_site")
        from trn_agent_boot.trn_boot import _ntff_profile_via_ctypes

        mod.set_axon_ntff_profile_hook(
            _ntff_profile_via_ctypes("/opt/axon/libaxon_pjrt.so")
        )
    except Exception:
        pass


def _move_gather_waits(nc, mybir):
    """InstDMAGatherAnt cannot carry sem waits on HW (custom NX decode path
    wedges the device).  Move each gather's on_wait onto a fresh Pool
    InstNoOp inserted immediately before it."""
    n_moved = 0
    for f in nc.m.functions:
        for blk in f.blocks:
            new_insts = []
            for inst in blk.instructions:
                if (
                    isinstance(inst, mybir.InstDMAGatherAnt)
                    and inst.sync_info
                    and inst.sync_info.on_wait
                ):
                    nop = mybir.InstNoOp(
                        name=f"I-gwaitc-{n_moved}",
                        ins=[],
                        outs=[],
                        engine=inst.engine,
                        sync_info=mybir.SyncInfo(
                            on_wait=list(inst.sync_info.on_wait), on_update=[]
                        ),
                        text_hint="gather_wait_carrier",
                        bass_nofuse=True,
                    )
                    inst.sync_info.on_wait.clear()
                    new_insts.append(nop)
                    n_moved += 1
                new_insts.append(inst)
            blk.instructions[:] = new_insts


def _wrap_idx(fi_pad):
    """[N] (mult of 16) row ids -> [128, N//16] int16 wrapped+replicated."""
    b16 = fi_pad.reshape(-1, 16)                  # [N/16, 16]
    return np.ascontiguousarray(b16[:, np.arange(P) % 16].T).astype(np.int16)


def _plan(flat_idx, seg):
    """Host-side gather/emission plan shared by all cores."""
    order = np.argsort(seg, kind="stable")
    fi = flat_idx[order].astype(np.int64)
    sg = seg[order].astype(np.int64)
    assert sg.min() >= 0 and sg.max() < B

    m = fi < SPLIT
    passes = []
    for fi_p, sg_p in ((fi[m], sg[m]), (fi[~m] - SPLIT, sg[~m])):
        n = len(fi_p)
        npad = (-n) % P
        fi_p = np.concatenate([fi_p, np.zeros(npad, np.int64)])
        sg_p = np.concatenate([sg_p, np.full(npad, PAD_SEG, np.int64)])
        ncols = len(fi_p) // P
        # column-major landing: call k slot t partition p holds token
        # k*2048 + t*128 + p  -> column j = k*16+t has tokens j*128 .. +127
        segcol = sg_p.reshape(ncols, P)
        passes.append(
            dict(idx=_wrap_idx(fi_p), segcol=segcol, ncols=ncols)
        )

    # emissions: (pass, col, window) -> one E-matmul each
    segadj_cols = []
    emissions_by_w = [[] for _ in range(NSEG_TILES)]
    for pi, pp in enumerate(passes):
        for j in range(pp["ncols"]):
            sc = pp["segcol"][j]
            valid = sc >= 0
            if not valid.any():
                continue
            w_first = int(sc[valid].min()) // P
            w_last = int(sc[valid].max()) // P
            assert w_last - w_first <= 1, (w_first, w_last)
            for w in range(w_first, w_last + 1):
                e = len(segadj_cols)
                adj = np.where(valid, sc - P * w, PAD_SEG)
                segadj_cols.append(adj.astype(np.float32))
                emissions_by_w[w].append((pi, j, e))
    segadjT = np.ascontiguousarray(np.stack(segadj_cols, axis=1))  # [128, n_em]
    for w in range(NSEG_TILES):
        assert emissions_by_w[w], f"seg tile {w} has no contributions"
    return passes, segadjT, emissions_by_w


def _build_program(passes, n_em, emissions_by_w):
    from concourse import bass, bacc, mybir
    import concourse.tile as tile

    f32 = mybir.dt.float32
    i16 = mybir.dt.int16

    ncols_lo, ncols_hi = passes[0]["ncols"], passes[1]["ncols"]

    nc = bacc.Bacc(num_swdge_queues=4)
    tbl_lo = nc.dram_tensor("tbl_lo", [VLO, DSH], f32, kind="ExternalInput")
    tbl_hi = nc.dram_tensor("tbl_hi", [VHI, DSH], f32, kind="ExternalInput")
    idx_lo = nc.dram_tensor("idx_lo", [P, ncols_lo * 8], i16, kind="ExternalInput")
    idx_hi = nc.dram_tensor("idx_hi", [P, ncols_hi * 8], i16, kind="ExternalInput")
    segadj = nc.dram_tensor("segadj", [P, n_em], f32, kind="ExternalInput")
    iota_d = nc.dram_tensor("iota", [P, P], f32, kind="ExternalInput")
    ident_d = nc.dram_tensor("ident", [P, P], f32, kind="ExternalInput")
    recip_d = nc.dram_tensor("recip", [NSEG_TILES, P], f32, kind="ExternalInput")
    wpack_d = nc.dram_tensor("wpack", [P, 4 * DE], f32, kind="ExternalInput")
    out_d = nc.dram_tensor("out", [B, DE], f32, kind="ExternalOutput")

    tbls = [tbl_lo, tbl_hi]
    idx_ds = [idx_lo, idx_hi]
    chunks = [(0, 128), (128, 128), (256, 128), (384, 64)]

    with tile.TileContext(nc) as tc:
        with (
            tc.tile_pool(name="const", bufs=1) as cpool,
            tc.tile_pool(name="g", bufs=4) as gpool,
            tc.tile_pool(name="e", bufs=4) as epool,
            tc.tile_pool(name="s", bufs=2) as spool,
            tc.tile_pool(name="mt", bufs=2) as mtpool,
            tc.tile_pool(name="osb", bufs=2) as opool,
            tc.tile_pool(name="rc", bufs=2) as rcpool,
            tc.tile_pool(name="pseg", bufs=2, space="PSUM") as pseg_pool,
            tc.tile_pool(name="pt", bufs=2, space="PSUM") as pt_pool,
            tc.tile_pool(name="po", bufs=2, space="PSUM") as po_pool,
        ):
            idx_sb = []
            for pi in range(2):
                t = cpool.tile([P, [ncols_lo, ncols_hi][pi] * 8], i16,
                               tag=f"idx{pi}")
                nc.sync.dma_start(out=t[:], in_=idx_ds[pi][:])
                idx_sb.append(t)
            segadj_sb = cpool.tile([P, n_em], f32, tag="segadj")
            nc.sync.dma_start(out=segadj_sb[:], in_=segadj[:])
            iota_sb = cpool.tile([P, P], f32, tag="iota")
            nc.sync.dma_start(out=iota_sb[:], in_=iota_d[:])
            ident_sb = cpool.tile([P, P], f32, tag="ident")
            nc.sync.dma_start(out=ident_sb[:], in_=ident_d[:])
            wpack_sb = cpool.tile([P, 4 * DE], f32, tag="wpack")
            nc.sync.dma_start(out=wpack_sb[:], in_=wpack_d[:])

            gtiles = {}
            qctr = [0]

            def get_gtile(pi, k):
                if (pi, k) in gtiles:
                    return gtiles[(pi, k)]
                ncols = passes[pi]["ncols"]
                c0 = k * CALL_COLS
                ncall = min(CALL_COLS, ncols - c0)
                g = gpool.tile([P, CALL_COLS, DSH], f32, tag="g")
                nc.gpsimd.dma_gather(
                    out_ap=g[:, :ncall, :],
                    in_ap=tbls[pi][:],
                    idxs_ap=idx_sb[pi][:, c0 * 8:(c0 + ncall) * 8],
                    num_idxs=ncall * P,
                    num_idxs_reg=ncall * P,
                    elem_size=DSH,
                    single_packet=False,
                    queue_num=qctr[0] % 4,
                )
                qctr[0] += 1
                gtiles[(pi, k)] = g
                return g

            for w in range(NSEG_TILES):
                ems = emissions_by_w[w]
                psum_seg = pseg_pool.tile([P, DSH], f32)
                for i, (pi, j, e) in enumerate(ems):
                    k, t = divmod(j, CALL_COLS)
                    g = get_gtile(pi, k)
                    E = epool.tile([P, P], f32, tag="E")
                    nc.vector.tensor_tensor(
                        out=E[:],
                        in0=segadj_sb[:, e:e + 1].to_broadcast([P, P]),
                        in1=iota_sb[:],
                        op=mybir.AluOpType.is_equal,
                    )
                    nc.tensor.matmul(
                        out=psum_seg[:],
                        lhsT=E[:],
                        rhs=g[:, t, :],
                        start=(i == 0),
                        stop=(i == len(ems) - 1),
                    )

                rc = rcpool.tile([P, 1], f32, tag="rc")
                nc.sync.dma_start(out=rc[:], in_=recip_d[w, :, None])
                s = spool.tile([P, DSH], f32, tag="s")
                nc.vector.tensor_scalar_mul(out=s[:], in0=psum_seg[:],
                                            scalar1=rc[:, :1])

                psum_o = po_pool.tile([P, DE], f32)
                for ci, (c0, clen) in enumerate(chunks):
                    psum_t = pt_pool.tile([P, P], f32, tag="pt")
                    nc.tensor.matmul(
                        out=psum_t[:clen, :],
                        lhsT=s[:, c0:c0 + clen],
                        rhs=ident_sb[:],
                        start=True,
                        stop=True,
                    )
                    mt = mtpool.tile([P, P], f32, tag="mt")
                    nc.vector.tensor_copy(out=mt[:clen, :], in_=psum_t[:clen, :])
                    nc.tensor.matmul(
                        out=psum_o[:],
                        lhsT=mt[:clen, :],
                        rhs=wpack_sb[:clen, ci * DE:(ci + 1) * DE],
                        start=(ci == 0),
                        stop=(ci == 3),
                    )
                osb = opool.tile([P, DE], f32, tag="osb")
                nc.vector.tensor_copy(out=osb[:], in_=psum_o[:])
                nc.sync.dma_start(out=out_d[w * P:(w + 1) * P, :], in_=osb[:])

    nc.compile()
    _move_gather_waits(nc, mybir)
    return nc


def kernel(flat_idx, seg, lens, embed_weight, proj_w, proj_b):
    global LAST_RESULTS
    _ensure_axon_ntff_hook()
    from concourse.bass_utils import run_bass_kernel_spmd

    flat_idx = np.asarray(flat_idx)
    seg = np.asarray(seg)
    lens = np.asarray(lens)
    embed_weight = np.asarray(embed_weight, dtype=np.float32)
    proj_w = np.asarray(proj_w, dtype=np.float32)
    proj_b = np.asarray(proj_b, dtype=np.float32)

    passes, segadjT, emissions_by_w = _plan(flat_idx, seg)
    n_em = segadjT.shape[1]

    nc = _build_program(passes, n_em, emissions_by_w)

    iota = np.tile(np.arange(P, dtype=np.float32), (P, 1))
    ident = np.eye(P, dtype=np.float32)
    recip = (1.0 / lens.astype(np.float64)).astype(np.float32).reshape(
        NSEG_TILES, P)

    in_maps = []
    for c in range(NCORES):
        sl = slice(c * DSH, (c + 1) * DSH)
        wc = proj_w[sl, :]                                 # [448, 128]
        wpack = np.zeros((P, 4 * DE), dtype=np.float32)
        for ci in range(4):
            r0, r1 = ci * P, min((ci + 1) * P, DSH)
            wpack[:r1 - r0, ci * DE:(ci + 1) * DE] = wc[r0:r1, :]
        in_maps.append({
            "tbl_lo": np.ascontiguousarray(embed_weight[:SPLIT, sl]),
            "tbl_hi": np.ascontiguousarray(embed_weight[SPLIT:, sl]),
            "idx_lo": passes[0]["idx"],
            "idx_hi": passes[1]["idx"],
            "segadj": segadjT,
            "iota": iota,
            "ident": ident,
            "recip": recip,
            "wpack": wpack,
        })

    res = run_bass_kernel_spmd(nc, in_maps, core_ids=list(range(NCORES)))
    LAST_RESULTS = res

    out = np.zeros((B, DE), dtype=np.float64)
    for c in range(NCORES):
        out += res.results[c]["out"].astype(np.float64)
    out += proj_b.astype(np.float64)
    return out.astype(np.float32)


# revision 9
# speedup vs baseline: 2.1468x; 2.1468x over previous
"""Trainium2 Bass kernel for nn_Aligner segment_reduce.

Computation: out = (segment_sum(embed_weight[flat_idx]) / lens) @ proj_w + proj_b
Shapes: flat_idx [65536], seg [65536] (sorted), lens [2048],
        embed_weight [50000, 3584], proj_w [3584, 128], proj_b [128].

Strategy (8 NeuronCores, data-parallel over segment-sorted tokens):
- Tokens are stably sorted by segment id; core k owns segments
  [256k, 256k+256) == two aligned 128-segment windows and gathers the
  full 3584-wide embedding rows of its tokens (fp16 copy of the table,
  ~62 MB of HBM reads per core).
- Vocab is split at 32768 so row ids fit int16 for the custom GPSIMD
  dma_gather instruction (two tables / two gather passes per window).
- Token sublists per (core, window, pass) are padded to global caps so
  every core runs the IDENTICAL program (SPMD) on different data.
- Each gather call fetches 4 columns x 128 rows.  Per 128-token column,
  an indicator matrix E[tok, seg_in_window] is built on-device
  (is_equal against an iota matrix, fp16) and E.T @ G accumulates
  segment sums for the window into a 7-bank [128, 3584] f32 PSUM tile.
- Per window: scale by 1/lens, transpose 128-wide chunks via matmul with
  identity, then GEMM with proj_w accumulating over the 28 chunks.
- Host assembles the per-core [256, 128] outputs and adds proj_b.
"""

import sys

sys.path.insert(0, "/opt/trn_rl_repo")

import numpy as np

T = 65536
B = 2048
V = 50000
D = 3584
DE = 128
NCORES = 8
SPLIT = 32768              # vocab split so indices fit int16
VLO, VHI = SPLIT, V - SPLIT
P = 128
CALL_COLS = 4              # 4 columns (512 indices) per dma_gather call
NW = 2                     # 128-seg windows per core
SEGS_PER_CORE = B // NCORES          # 256
NCHUNK = D // 512          # 7 psum-bank chunks for the E-matmuls
NTCH = D // P              # 28 transpose/proj chunks
PAD_SEG = -1000            # sentinel seg value for padded token slots

LAST_RESULTS = None        # BassKernelResults of the most recent run


def _ensure_axon_ntff_hook():
    """bass_utils imports antenv.axon_hooks when trace=True under axon;
    some images lack that module.  Provide it, wired to the libaxon ctypes
    NTFF profiler when available (else the hook stays None and bass_utils
    skips tracing gracefully)."""
    try:
        from antenv import axon_hooks  # noqa: F401
        return
    except ImportError:
        pass
    import types

    try:
        import antenv
    except ImportError:
        return
    mod = types.ModuleType("antenv.axon_hooks")
    _hook = [None]
    mod.set_axon_ntff_profile_hook = lambda h: _hook.__setitem__(0, h)
    mod.get_axon_ntff_profile_hook = lambda: _hook[0]
    sys.modules["antenv.axon_hooks"] = mod
    antenv.axon_hooks = mod
    try:
        if "/root/.axon_site" not in sys.path:
            sys.path.insert(0, "/root/.axon_site")
        from trn_agent_boot.trn_boot import _ntff_profile_via_ctypes

        mod.set_axon_ntff_profile_hook(
            _ntff_profile_via_ctypes("/opt/axon/libaxon_pjrt.so")
        )
    except Exception:
        pass


def _move_gather_waits(nc, mybir):
    """InstDMAGatherAnt cannot carry sem waits on HW (custom NX decode path
    wedges the device).  Move each gather's on_wait onto a fresh Pool
    InstNoOp inserted immediately before it."""
    n_moved = 0
    for f in nc.m.functions:
        for blk in f.blocks:
            new_insts = []
            for inst in blk.instructions:
                if (
                    isinstance(inst, mybir.InstDMAGatherAnt)
                    and inst.sync_info
                    and inst.sync_info.on_wait
                ):
                    nop = mybir.InstNoOp(
                        name=f"I-gwaitc-{n_moved}",
                        ins=[],
                        outs=[],
                        engine=inst.engine,
                        sync_info=mybir.SyncInfo(
                            on_wait=list(inst.sync_info.on_wait), on_update=[]
                        ),
                        text_hint="gather_wait_carrier",
                        bass_nofuse=True,
                    )
                    inst.sync_info.on_wait.clear()
                    new_insts.append(nop)
                    n_moved += 1
                new_insts.append(inst)
            blk.instructions[:] = new_insts


def _wrap_idx(fi_pad):
    """[N] (mult of 16) row ids -> [128, N//16] int16 wrapped+replicated."""
    b16 = fi_pad.reshape(-1, 16)                  # [N/16, 16]
    return np.ascontiguousarray(b16[:, np.arange(P) % 16].T).astype(np.int16)


def _plan(flat_idx, seg):
    """Host-side plan: per-(core, window, pass) token sublists padded to
    global caps so the device program is core-independent."""
    order = np.argsort(seg, kind="stable")
    fi = flat_idx[order].astype(np.int64)
    sg = seg[order].astype(np.int64)
    assert sg.min() >= 0 and sg.max() < B

    lo_mask = fi < SPLIT
    # token sublists for (core k, window w, pass p)
    sub = {}
    counts = np.zeros((NCORES, NW, 2), dtype=np.int64)
    wslot = sg // P                  # global 128-seg window 0..15
    for k in range(NCORES):
        for w in range(NW):
            in_win = wslot == (k * NW + w)
            for p in range(2):
                m = in_win & (lo_mask if p == 0 else ~lo_mask)
                f = fi[m] - (0 if p == 0 else SPLIT)
                s = sg[m] - (k * SEGS_PER_CORE + w * P)   # 0..127
                sub[(k, w, p)] = (f, s)
                counts[k, w, p] = len(f)

    caps = []
    for p in range(2):
        cap = int(counts[:, :, p].max())
        cap = -(-cap // P) * P                    # round up to 128
        caps.append(cap)

    # per-core packed index + segadj arrays
    idx_arrs = [[], []]          # per pass: list per core of wrapped idx
    segadj_arrs = []             # per core: [128, ncols_total] fp16
    ncols = [caps[0] // P, caps[1] // P]
    for k in range(NCORES):
        w_idx = [[], []]
        adj_cols = []
        for w in range(NW):
            for p in range(2):
                f, s = sub[(k, w, p)]
                npad = caps[p] - len(f)
                f = np.concatenate([f, np.zeros(npad, np.int64)])
                s = np.concatenate([s, np.full(npad, PAD_SEG, np.int64)])
                w_idx[p].append(f)
                adj_cols.append(s.reshape(ncols[p], P))
        for p in range(2):
            idx_arrs[p].append(_wrap_idx(np.concatenate(w_idx[p])))
        # segadj layout matches emission order: w0:[A cols..B cols] w1:[...]
        adj = np.concatenate(
            [adj_cols[0], adj_cols[1], adj_cols[2], adj_cols[3]], axis=0
        )                                           # [ncols_total, 128]
        segadj_arrs.append(
            np.ascontiguousarray(adj.T.astype(np.float16))
        )
    return idx_arrs, segadj_arrs, ncols


def _calls(ncol):
    """Split ncol columns into dma_gather calls of <= CALL_COLS columns."""
    out = []
    c = 0
    while c < ncol:
        n = min(CALL_COLS, ncol - c)
        out.append((c, n))
        c += n
    return out


def _build_program(ncols):
    from concourse import bass, bacc, mybir
    import concourse.tile as tile

    f32 = mybir.dt.float32
    f16 = mybir.dt.float16
    i16 = mybir.dt.int16

    ncols_a, ncols_b = ncols
    ncols_tot = NW * (ncols_a + ncols_b)

    nc = bacc.Bacc()
    tbl_lo = nc.dram_tensor("tbl_lo", [VLO, D], f16, kind="ExternalInput")
    tbl_hi = nc.dram_tensor("tbl_hi", [VHI, D], f16, kind="ExternalInput")
    idx_lo = nc.dram_tensor("idx_lo", [P, NW * ncols_a * 8], i16,
                            kind="ExternalInput")
    idx_hi = nc.dram_tensor("idx_hi", [P, NW * ncols_b * 8], i16,
                            kind="ExternalInput")
    segadj = nc.dram_tensor("segadj", [P, ncols_tot], f16, kind="ExternalInput")
    iota_d = nc.dram_tensor("iota", [P, CALL_COLS * P], f16,
                            kind="ExternalInput")
    ident_d = nc.dram_tensor("ident", [P, P], f32, kind="ExternalInput")
    recip_d = nc.dram_tensor("recip", [NW, P], f32, kind="ExternalInput")
    wpack_d = nc.dram_tensor("wpack", [P, NTCH * DE], f32, kind="ExternalInput")
    out_d = nc.dram_tensor("out", [SEGS_PER_CORE, DE], f32,
                           kind="ExternalOutput")
    import os
    dbg = os.environ.get("KDBG") == "1"
    if dbg:
        dbg_d = nc.dram_tensor("dbg_s", [SEGS_PER_CORE, D], f32,
                               kind="ExternalOutput")

    tbls = [tbl_lo, tbl_hi]
    idx_ds = [idx_lo, idx_hi]

    with tile.TileContext(nc) as tc:
        with (
            tc.tile_pool(name="const", bufs=1) as cpool,
            tc.tile_pool(name="g", bufs=3) as gpool,
            tc.tile_pool(name="e", bufs=3) as epool,
            tc.tile_pool(name="s", bufs=2) as spool,
            tc.tile_pool(name="mt", bufs=2) as mtpool,
            tc.tile_pool(name="osb", bufs=2) as opool,
            tc.tile_pool(name="rc", bufs=2) as rcpool,
            tc.tile_pool(name="pseg", bufs=1, space="PSUM") as pseg_pool,
            tc.tile_pool(name="pc", bufs=1, space="PSUM") as pc_pool,
        ):
            idx_sb = []
            for p in range(2):
                t = cpool.tile([P, NW * ncols[p] * 8], i16, tag=f"idx{p}")
                nc.sync.dma_start(out=t[:], in_=idx_ds[p][:])
                idx_sb.append(t)
            segadj_sb = cpool.tile([P, ncols_tot], f16, tag="segadj")
            nc.sync.dma_start(out=segadj_sb[:], in_=segadj[:])
            iota_sb = cpool.tile([P, CALL_COLS, P], f16, tag="iota")
            nc.sync.dma_start(
                out=iota_sb[:],
                in_=iota_d[:].rearrange("p (a b) -> p a b", a=CALL_COLS),
            )
            ident_sb = cpool.tile([P, P], f32, tag="ident")
            nc.sync.dma_start(out=ident_sb[:], in_=ident_d[:])
            wpack_sb = cpool.tile([P, NTCH * DE], f32, tag="wpack")
            nc.sync.dma_start(out=wpack_sb[:], in_=wpack_d[:])

            ecol = [0]
            for w in range(NW):
                pseg = pseg_pool.tile([P, D], f32, tag="pseg")
                n_win_cols = ncols_a + ncols_b
                col_in_win = 0
                for p in range(2):
                    npc = ncols[p]
                    for (c0, ncall) in _calls(npc):
                        g = gpool.tile([P, CALL_COLS, D], f16, tag="g")
                        icol0 = (w * npc + c0) * 8
                        nc.gpsimd.dma_gather(
                            out_ap=g[:, :ncall, :],
                            in_ap=tbls[p][:],
                            idxs_ap=idx_sb[p][:, icol0:icol0 + ncall * 8],
                            num_idxs=ncall * P,
                            num_idxs_reg=ncall * P,
                            elem_size=D,
                            single_packet=False,
                        )
                        e0 = ecol[0]
                        E = epool.tile([P, CALL_COLS, P], f16, tag="E")
                        nc.vector.tensor_tensor(
                            out=E[:, :ncall, :],
                            in0=segadj_sb[:, e0:e0 + ncall].to_broadcast(
                                [P, ncall, P]),
                            in1=iota_sb[:, :ncall, :],
                            op=mybir.AluOpType.is_equal,
                        )
                        ecol[0] += ncall
                        for t in range(ncall):
                            for ch in range(NCHUNK):
                                nc.tensor.matmul(
                                    out=pseg[:, ch * 512:(ch + 1) * 512],
                                    lhsT=E[:, t, :],
                                    rhs=g[:, t, ch * 512:(ch + 1) * 512],
                                    start=(col_in_win == 0),
                                    stop=(col_in_win == n_win_cols - 1),
                                )
                            col_in_win += 1

                rc = rcpool.tile([P, 1], f32, tag="rc")
                nc.sync.dma_start(out=rc[:], in_=recip_d[w, :, None])
                s = spool.tile([P, D], f32, tag="s")
                nc.vector.tensor_scalar_mul(out=s[:], in0=pseg[:],
                                            scalar1=rc[:, :1])
                if dbg:
                    nc.sync.dma_start(out=dbg_d[w * P:(w + 1) * P, :], in_=s[:])

                po = pseg_pool.tile([P, DE], f32, tag="pseg")
                for ci in range(NTCH):
                    pt = pc_pool.tile([P, P], f32, tag="pt")
                    nc.tensor.matmul(
                        out=pt[:],
                        lhsT=s[:, ci * P:(ci + 1) * P],
                        rhs=ident_sb[:],
                        start=True,
                        stop=True,
                    )
                    mt = mtpool.tile([P, P], f32, tag="mt")
                    nc.scalar.copy(out=mt[:], in_=pt[:])
                    nc.tensor.matmul(
                        out=po[:],
                        lhsT=mt[:],
                        rhs=wpack_sb[:, ci * DE:(ci + 1) * DE],
                        start=(ci == 0),
                        stop=(ci == NTCH - 1),
                    )
                osb = opool.tile([P, DE], f32, tag="osb")
                nc.vector.tensor_copy(out=osb[:], in_=po[:])
                nc.sync.dma_start(out=out_d[w * P:(w + 1) * P, :], in_=osb[:])

    nc.compile()
    _move_gather_waits(nc, mybir)
    return nc


def kernel(flat_idx, seg, lens, embed_weight, proj_w, proj_b):
    global LAST_RESULTS
    _ensure_axon_ntff_hook()
    from concourse.bass_utils import run_bass_kernel_spmd

    flat_idx = np.asarray(flat_idx)
    seg = np.asarray(seg)
    lens = np.asarray(lens)
    embed_weight = np.asarray(embed_weight, dtype=np.float32)
    proj_w = np.asarray(proj_w, dtype=np.float32)
    proj_b = np.asarray(proj_b, dtype=np.float32)

    idx_arrs, segadj_arrs, ncols = _plan(flat_idx, seg)
    nc = _build_program(ncols)

    emb16 = embed_weight.astype(np.float16)
    tbl_lo = np.ascontiguousarray(emb16[:SPLIT])
    tbl_hi = np.ascontiguousarray(emb16[SPLIT:])

    iota = np.tile(np.arange(P, dtype=np.float16), (P, CALL_COLS))
    ident = np.eye(P, dtype=np.float32)
    recip_all = (1.0 / lens.astype(np.float64)).astype(np.float32).reshape(
        B // P, P)
    wpack = np.ascontiguousarray(
        proj_w.reshape(NTCH, P, DE).transpose(1, 0, 2).reshape(P, NTCH * DE))
    # wpack[r, ci*DE + e] = proj_w[ci*128 + r, e]

    in_maps = []
    for k in range(NCORES):
        in_maps.append({
            "tbl_lo": tbl_lo,
            "tbl_hi": tbl_hi,
            "idx_lo": idx_arrs[0][k],
            "idx_hi": idx_arrs[1][k],
            "segadj": segadj_arrs[k],
            "iota": iota,
            "ident": ident,
            "recip": recip_all[k * NW:(k + 1) * NW],
            "wpack": wpack,
        })

    res = run_bass_kernel_spmd(nc, in_maps, core_ids=list(range(NCORES)))
    LAST_RESULTS = res

    out = np.empty((B, DE), dtype=np.float32)
    for k in range(NCORES):
        out[k * SEGS_PER_CORE:(k + 1) * SEGS_PER_CORE, :] = (
            res.results[k]["out"])
    out += proj_b
    return out
